# revision 26
# baseline (speedup 1.0000x reference)
"""Trainium2 Bass kernel for nn_H2GT_HGNN: two HGNN convs + single-head GAT +
gated attention pooling, sharded across 8 NeuronCores.

Strategy (dst-node row sharding, core c owns rows R_c = c*1024..(c+1)*1024):
  - Host passes G^T (bf16) so both G @ X convs read contiguous column blocks.
    G^T block is loaded ONCE into SBUF (16 MB resident) and reused by both
    convs; the edge-mask C block streams during the attention phase only.
  - GAT edge softmax is reformulated dense + per-node-rank-1.  Dividing the
    softmax numerator and denominator by ev_v = exp(ed_v) (cancels):
        w[u,v] = C[u,v] * max(eu_u, eu2_u * p_v)
    with eu=exp(es), eu2=exp(0.2 es), p=exp(-0.8 ed).  Per u-chunk this is
    ONE tensor_scalar (4x DVE mode) + ONE tensor_tensor (2x mode), then a
    transposed matmul attT += zchunk^T @ w (stationary z, moving w) plus an
    identity-matmul S += I^T @ w whose column sums give the denominator.
  - Collectives: tiny warmup AllGather first (absorbs comm init), AllGather
    of XW1 / HW2 / [z rows | es/ed rows], final AllReduce of [1,129].
"""

import numpy as np
import ml_dtypes

BF16 = ml_dtypes.bfloat16

# Problem sizes (hardcoded per contract)
N, F_IN, HID, OUT, D_ATT, E = 8192, 512, 256, 128, 64, 262144
M = 8          # cores
P = 128        # partitions
R = N // M     # rows per core (1024)
K = N // P     # contraction chunks (64)
RT = R // P    # row tiles per core (8)
O1 = OUT + 1   # out plus ones column (pooling)
GRP = 4        # chunks per big-matrix DMA group
NG = K // GRP  # groups (16)
ZR = R + 2 * RT  # zeb rows: R z-rows + 16 es/ed rows (1040)


def build_program():
    import concourse.bass as bass
    import concourse.tile as tile
    from concourse import bacc, mybir
    from contextlib import ExitStack

    dt = mybir.dt
    ALU = mybir.AluOpType
    ACT = mybir.ActivationFunctionType

    nc = bacc.Bacc(None, target_bir_lowering=False, debug=False)

    # ---- I/O ----
    gt = nc.dram_tensor("gt", [N, R], dt.bfloat16, kind="ExternalInput")
    ct = nc.dram_tensor("ct", [N, R], dt.bfloat16, kind="ExternalInput")
    xt = nc.dram_tensor("xt", [F_IN, R], dt.bfloat16, kind="ExternalInput")
    w1 = nc.dram_tensor("w1", [F_IN, HID], dt.bfloat16, kind="ExternalInput")
    w2 = nc.dram_tensor("w2", [HID, OUT], dt.bfloat16, kind="ExternalInput")
    wgv = nc.dram_tensor("wgv", [OUT, OUT + 2], dt.bfloat16, kind="ExternalInput")
    wab = nc.dram_tensor("wab", [OUT, 2 * D_ATT], dt.bfloat16, kind="ExternalInput")
    b1bc = nc.dram_tensor("b1bc", [P, HID], dt.float32, kind="ExternalInput")
    b2bc = nc.dram_tensor("b2bc", [P, OUT], dt.float32, kind="ExternalInput")
    babbc = nc.dram_tensor("babbc", [P, 2 * D_ATT], dt.float32, kind="ExternalInput")
    wcbc = nc.dram_tensor("wcbc", [P, D_ATT], dt.float32, kind="ExternalInput")
    bcbc = nc.dram_tensor("bcbc", [P, 1], dt.float32, kind="ExternalInput")
    ident = nc.dram_tensor("ident", [P, P], dt.bfloat16, kind="ExternalInput")
    colsel = nc.dram_tensor("colsel", [P, RT * RT], dt.bfloat16, kind="ExternalInput")
    out_ext = nc.dram_tensor("out", [1, OUT], dt.float32, kind="ExternalOutput")

    groups = [list(range(M))]

    with tile.TileContext(nc) as tc, ExitStack() as ctx:
        dram = ctx.enter_context(tc.tile_pool(name="dram", bufs=1, space="DRAM"))
        xw1b = dram.tile([R, HID], dt.bfloat16)
        xw1f = dram.tile([N, HID], dt.bfloat16, addr_space="Shared")
        hw2b = dram.tile([R, OUT], dt.bfloat16)
        hw2f0 = dram.tile([N // 2, OUT], dt.bfloat16, addr_space="Shared")
        hw2f1 = dram.tile([N // 2, OUT], dt.bfloat16, addr_space="Shared")
        zeb = dram.tile([ZR, OUT], dt.bfloat16)
        zef = dram.tile([M * ZR, OUT], dt.bfloat16, addr_space="Shared")
        poolb = dram.tile([1, O1], dt.float32)
        poolr = dram.tile([1, O1], dt.float32, addr_space="Shared")

        const = ctx.enter_context(tc.tile_pool(name="const", bufs=1))
        big = ctx.enter_context(tc.tile_pool(name="big", bufs=1))

        # ---- resident constants (x^T load is emitted first, inside the
        # gt-pool block below, so it leads the scalar DMA queue) ----
        w1_sb = const.tile([P, (F_IN // P) * HID], dt.bfloat16)
        nc.scalar.dma_start(w1_sb.rearrange("p (k h) -> p k h", h=HID),
                            w1.rearrange("(k p) h -> p k h", p=P))
        w2_sb = const.tile([P, (HID // P) * OUT], dt.bfloat16)
        nc.scalar.dma_start(w2_sb.rearrange("p (k h) -> p k h", h=OUT),
                            w2.rearrange("(k p) h -> p k h", p=P))
        wgv_sb = const.tile([P, OUT + 2], dt.bfloat16)
        nc.scalar.dma_start(wgv_sb[:], wgv[:])
        wab_sb = const.tile([P, 2 * D_ATT], dt.bfloat16)
        nc.scalar.dma_start(wab_sb[:], wab[:])
        b1bc_sb = const.tile([P, HID], dt.float32)
        nc.scalar.dma_start(b1bc_sb[:], b1bc[:])
        b2bc_sb = const.tile([P, OUT], dt.float32)
        nc.scalar.dma_start(b2bc_sb[:], b2bc[:])
        babbc_sb = const.tile([P, 2 * D_ATT], dt.float32)
        nc.scalar.dma_start(babbc_sb[:], babbc[:])
        wcbc_sb = const.tile([P, D_ATT], dt.float32)
        nc.scalar.dma_start(wcbc_sb[:], wcbc[:])
        bcbc_sb = const.tile([P, 1], dt.float32)
        nc.scalar.dma_start(bcbc_sb[:], bcbc[:])
        ident_sb = const.tile([P, P], dt.bfloat16)
        nc.scalar.dma_start(ident_sb[:], ident[:])
        colsel_sb = const.tile([P, RT * RT], dt.bfloat16)
        nc.scalar.dma_start(colsel_sb[:], colsel[:])
        ones_sb = const.tile([1, P], dt.bfloat16)
        nc.vector.memset(ones_sb[:], 1.0)

        # ---- G^T resident load: 16 groups of 4 chunks (sync DMA queue).
        # The pool closes after phase B2 so the C-mask stream can reuse the
        # space during phase D.
        h2T_sb = big.tile([P, R], dt.bfloat16)
        hT_sb = big.tile([P, 2 * R], dt.bfloat16)  # [hid 2x128, r=1024]
        with tc.tile_pool(name="gtp", bufs=1) as gt_pool:
            xt_pool = tc.tile_pool(name="xtp", bufs=1)
            xt_po = xt_pool.__enter__()
            xt_sb = xt_po.tile([P, (F_IN // P) * R], dt.bfloat16)
            nc.scalar.dma_start(xt_sb.rearrange("p (k r) -> p k r", r=R),
                                xt.rearrange("(k p) r -> p k r", p=P))
            # Dummy read of the AG1 output gates the sync DMA queue: the
            # bulk G^T prefetch stays off the HBM bus until the first
            # collective (whose firmware init runs at a fixed early time)
            # completes -- concurrent bulk traffic stretches the init 3x.
            warm_sb = gt_pool.tile([M, 4], dt.bfloat16)
            nc.sync.dma_start(warm_sb[:], xw1f[0:M, 0:4])
            gt_g = []
            for g in range(NG):
                t = gt_pool.tile([P, GRP, R], dt.bfloat16, name=f"gt{g}")
                nc.sync.dma_start(
                    t[:], gt.rearrange("(g i p) r -> g p i r", i=GRP, p=P)[g])
                gt_g.append(t)

            # ============= phase A: XW1 rows R_c, then AllGather =============
            with tc.tile_pool(name="phA", bufs=1) as pa, \
                 tc.tile_pool(name="phA_ps", bufs=1, space="PSUM") as pa_ps:
                xw1_sb = pa.tile([P, RT, HID], dt.bfloat16)
                for rt in range(RT):
                    ps = pa_ps.tile([P, HID], dt.float32, tag="pa", bufs=2)
                    for k in range(F_IN // P):
                        nc.tensor.matmul(
                            ps[:],
                            xt_sb[:, k * R + rt * P : k * R + (rt + 1) * P],
                            w1_sb[:, k * HID : (k + 1) * HID],
                            start=(k == 0), stop=(k == F_IN // P - 1))
                    nc.vector.tensor_tensor(xw1_sb[:, rt, :], ps[:],
                                            b1bc_sb[:], op=ALU.add)
                nc.scalar.dma_start(
                    xw1b.rearrange("(rt p) h -> p rt h", p=P), xw1_sb[:])
            nc.gpsimd.collective_compute(
                "AllGather", ALU.bypass, replica_groups=groups,
                ins=[xw1b[:]], outs=[xw1f[:]])
            xt_pool.__exit__(None, None, None)

            # ============= phase B1: hT = relu(G @ XW1)^T ====================
            # Output-column split: the n=0 half of hT finishes first, so the
            # first half of HW2 (row tiles 0-3) and its AllGather run on the
            # CC queue while the PE computes the n=1 half.
            with tc.tile_pool(name="phB1", bufs=1) as pb, \
                 tc.tile_pool(name="phB1_ps", bufs=1, space="PSUM") as pb_ps, \
                 tc.tile_pool(name="phB1b_ps", bufs=1, space="PSUM") as pbb_ps:
                hps = [pb_ps.tile([P, 512], dt.float32, tag=f"h{j}{n}", bufs=1,
                                  name=f"hps{j}{n}")
                       for j in range(2) for n in range(2)]
                xw1cs = []
                for g in range(NG):
                    xw1c = pb.tile([P, GRP, HID], dt.bfloat16, name=f"xw1c{g}")
                    nc.scalar.dma_start(
                        xw1c[:],
                        xw1f.rearrange("(g i p) h -> g p i h", i=GRP, p=P)[g])
                    xw1cs.append(xw1c)
                hw2_sb = pb.tile([P, RT, OUT], dt.bfloat16)

                def b1_half(n):
                    for g in range(NG):
                        for i in range(GRP):
                            k = g * GRP + i
                            gsl = gt_g[g][:, i, :]
                            for j in range(2):
                                nc.tensor.matmul(
                                    hps[2 * j + n][:],
                                    xw1cs[g][:, i, j * P : (j + 1) * P],
                                    gsl[:, n * 512 : (n + 1) * 512],
                                    start=(k == 0), stop=(k == K - 1))
                    for j in range(2):
                        nc.scalar.activation(
                            hT_sb[:, j * R + n * 512 : j * R + (n + 1) * 512],
                            hps[2 * j + n][:], ACT.Relu)

                def hw2_part(half):
                    for rt in range(4 * half, 4 * half + 4):
                        ps2 = pbb_ps.tile([P, OUT], dt.float32, tag="hw2",
                                          bufs=2)
                        for j in range(2):
                            nc.tensor.matmul(
                                ps2[:],
                                hT_sb[:, j * R + rt * P : j * R + (rt + 1) * P],
                                w2_sb[:, j * OUT : (j + 1) * OUT],
                                start=(j == 0), stop=(j == 1))
                        nc.vector.tensor_tensor(hw2_sb[:, rt, :], ps2[:],
                                                b2bc_sb[:], op=ALU.add)
                    nc.scalar.dma_start(
                        hw2b.rearrange("(hf rt p) h -> hf p rt h",
                                       hf=2, p=P)[half],
                        hw2_sb[:, 4 * half : 4 * half + 4, :])

                hw2cs = {}

                def b2_dmas(half, hw2fh, pool):
                    for c in range(M):
                        hw2c = pool.tile([P, GRP, OUT], dt.bfloat16,
                                         name=f"hw2c{2 * c + half}")
                        nc.scalar.dma_start(
                            hw2c[:],
                            hw2fh.rearrange("(c i p) h -> c p i h",
                                            i=GRP, p=P)[c])
                        hw2cs[2 * c + half] = hw2c

                b1_half(0)
                hw2_part(0)
                nc.gpsimd.collective_compute(
                    "AllGather", ALU.bypass, replica_groups=groups,
                    ins=[hw2b[0 : R // 2, :]], outs=[hw2f0[:]])
                b2_dmas(0, hw2f0, pb)
                b1_half(1)
                hw2_part(1)
                nc.gpsimd.collective_compute(
                    "AllGather", ALU.bypass, replica_groups=groups,
                    ins=[hw2b[R // 2 : R, :]], outs=[hw2f1[:]])
                b2_dmas(1, hw2f1, pb)

                # ========= phase B2: h2T = (G @ HW2)^T =======================
                # Contraction split: u-chunks covered by hw2f0 (m=0..3 of
                # every core) are accumulated first; the second half-AllGather
                # hides under them.
                with tc.tile_pool(name="phB2_ps", bufs=1,
                                  space="PSUM") as pc_ps:
                    h2ps = [pc_ps.tile([P, 512], dt.float32, tag=f"h2{n}",
                                       bufs=1, name=f"h2ps{n}")
                            for n in range(2)]
                    for half in range(2):
                        for c in range(M):
                            g = 2 * c + half
                            for i in range(GRP):
                                gsl = gt_g[g][:, i, :]
                                for n in range(2):
                                    nc.tensor.matmul(
                                        h2ps[n][:], hw2cs[g][:, i, :],
                                        gsl[:, n * 512 : (n + 1) * 512],
                                        start=(half == 0 and c == 0
                                               and i == 0),
                                        stop=(half == 1 and c == M - 1
                                              and i == GRP - 1))
                    for n in range(2):
                        nc.scalar.activation(
                            h2T_sb[:, n * 512 : (n + 1) * 512], h2ps[n][:],
                            ACT.Copy)

        # ---- z rows + es/ed, packed into one AllGather buffer ----
        esed_col = big.tile([P, 2 * RT], dt.bfloat16)  # [v-part, (rt: es, ed)]
        with tc.tile_pool(name="phB2b", bufs=1) as pd, \
             tc.tile_pool(name="phB2b_ps", bufs=1, space="PSUM") as pd_ps:
            zrow = pd.tile([P, RT, OUT], dt.bfloat16)
            for rt in range(RT):
                zps = pd_ps.tile([P, OUT + 2], dt.float32, tag="z", bufs=2)
                nc.tensor.matmul(
                    zps[:], h2T_sb[:, rt * P : (rt + 1) * P], wgv_sb[:],
                    start=True, stop=True)
                nc.scalar.activation(zrow[:, rt, :], zps[:, 0:OUT], ACT.Copy)
                nc.vector.tensor_copy(esed_col[:, 2 * rt : 2 * rt + 2],
                                      zps[:, OUT : OUT + 2])
            nc.scalar.dma_start(
                zeb[0:R, :].rearrange("(rt p) h -> p rt h", p=P), zrow[:])
            # es/ed rows (local, transposed) appended below the z rows
            eT_ps = pd_ps.tile([2 * RT, P], dt.bfloat16, tag="eT", bufs=1)
            nc.tensor.transpose(eT_ps[:], esed_col[:], ident_sb[:])
            eT_sb = pd.tile([2 * RT, P], dt.bfloat16)
            nc.scalar.activation(eT_sb[:], eT_ps[:], ACT.Copy)
            nc.scalar.dma_start(zeb[R : ZR, :], eT_sb[:])

        # p_bc[u, v] = exp(-0.8 * ed_v): built from LOCAL ed only, emitted
        # before the AllGathers so it overlaps them.
        p_bc = big.tile([P, R], dt.bfloat16)
        with tc.tile_pool(name="phPb", bufs=1) as pe0, \
             tc.tile_pool(name="phPb_ps", bufs=1, space="PSUM") as pe0_ps:
            for m in range(RT):
                edr_ps = pe0_ps.tile([1, P], dt.bfloat16, tag="edr", bufs=2)
                nc.tensor.transpose(
                    edr_ps[:], esed_col[:, 2 * m + 1 : 2 * m + 2], ident_sb[:])
                edr_sb = pe0.tile([1, P], dt.bfloat16, tag="edrs", bufs=2)
                nc.vector.tensor_copy(edr_sb[:], edr_ps[:])
                bc_ps = pe0_ps.tile([P, P], dt.float32, tag="bc", bufs=2)
                nc.tensor.matmul(bc_ps[:], ones_sb[:], edr_sb[:],
                                 start=True, stop=True)
                nc.scalar.activation(
                    p_bc[:, m * P : (m + 1) * P], bc_ps[:], ACT.Exp, scale=-0.8)

        nc.gpsimd.collective_compute(
            "AllGather", ALU.bypass, replica_groups=groups,
            ins=[zeb[:]], outs=[zef[:]])

        zef_r = zef.rearrange("(c q) f -> c q f", q=ZR)

        # ================= phase D prep ======================================
        # eu_all/eu2_all hold exp(es), exp(0.2 es) for ALL nodes in columns
        # m*16 + 2c (odd columns hold exp of ed -- unused junk).
        eu_all = big.tile([P, 2 * K], dt.float32)
        eu2_all = big.tile([P, 2 * K], dt.float32)
        with tc.tile_pool(name="phDp", bufs=1) as pe, \
             tc.tile_pool(name="phDp_ps", bufs=1, space="PSUM") as pe_ps:
            # eu/eu2 for all nodes: one gather of every es/ed row, 8 packed
            # transposes into one PSUM tile, then two full-width exps.
            est_all = pe.tile([2 * M, RT * P], dt.bfloat16)
            for m in range(RT):
                nc.scalar.dma_start(
                    est_all[:, m * P : (m + 1) * P]
                    .rearrange("(c j) f -> c j f", j=2),
                    zef_r[:, R + 2 * m : R + 2 * m + 2, :])
            estT_ps = pe_ps.tile([P, 2 * K], dt.bfloat16, tag="estT", bufs=1)
            for m in range(RT):
                nc.tensor.transpose(
                    estT_ps[:, 2 * M * m : 2 * M * (m + 1)],
                    est_all[:, m * P : (m + 1) * P],
                    ident_sb[0:2 * M, 0:2 * M])
            nc.scalar.activation(eu_all[:], estT_ps[:], ACT.Exp)
            nc.scalar.activation(eu2_all[:], estT_ps[:], ACT.Exp, scale=0.2)

        # z chunks for the attention stationaries
        zfc_pool = ctx.enter_context(tc.tile_pool(name="zfcp", bufs=1))
        zfcs = {}
        for g in [2 * c + half for half in range(2) for c in range(M)]:
            half, c = g % 2, g // 2
            zfc = zfc_pool.tile([P, GRP, OUT], dt.bfloat16, name=f"zfc{g}")
            nc.scalar.dma_start(
                zfc[:],
                zef_r[c, half * GRP * P : (half * GRP + GRP) * P, :]
                .rearrange("(i p) f -> p i f", p=P))
            zfcs[g] = zfc

        # C mask groups stream into the space freed by gt (sync queue);
        # gated behind the last AllGather (dummy read of its output) so their
        # HBM traffic cannot slow the collectives.
        ct_pool = ctx.enter_context(tc.tile_pool(name="ctp", bufs=1))
        ct_gate = ct_pool.tile([M, 4], dt.bfloat16)
        nc.sync.dma_start(ct_gate[:], zef[0:M, 0:4])
        ct_g = [None] * NG
        for g in [2 * c + half for half in range(2) for c in range(M)]:
            t = ct_pool.tile([P, GRP, R], dt.bfloat16, name=f"ct{g}")
            nc.sync.dma_start(
                t[:], ct.rearrange("(g i p) r -> g p i r", i=GRP, p=P)[g])
            ct_g[g] = t

        # ================= phase D loop ======================================
        # DVE computes t (tensor_scalar, 2x mode) then the mask product
        # (tensor_tensor, 2x mode); GPSIMD is deliberately NOT used -- a
        # concurrent GpSimd op halves DVE throughput (SBUF contention).
        # Matmuls are issued in reversed order within groups of GRP chunks
        # so the PE fires 16 back-to-back matmuls per group (a >=3.4us
        # burst holds the HAM clock warm).
        attU_sb = big.tile([P, R], dt.bfloat16)   # relu(att)^T  [f, v]
        S_sb = big.tile([P, R], dt.bfloat16)      # sum_k w_k    [p, v]
        with tc.tile_pool(name="phD", bufs=1) as pf, \
             tc.tile_pool(name="phD_ps", bufs=1, space="PSUM") as pf_ps:
            att_ps = [pf_ps.tile([P, 512], dt.float32, tag=f"att{n}", bufs=1,
                                 name=f"attps{n}") for n in range(2)]
            s_ps = [pf_ps.tile([P, 512], dt.float32, tag=f"s{n}", bufs=1,
                               name=f"sps{n}") for n in range(2)]
            gorder = [2 * c + half for half in range(2) for c in range(M)]
            korder = []
            for g in gorder:
                korder.extend(reversed(range(g * GRP, (g + 1) * GRP)))
            kfirst, klast = korder[0], korder[-1]
            for g in gorder:
                zfc = zfcs[g]
                t_t = pf.tile([P, GRP * R], dt.bfloat16, tag="tt", bufs=2)
                for i2 in range(GRP):
                    k = g * GRP + i2
                    c, m = k // RT, k % RT
                    ecol = 2 * M * m + 2 * c  # exp(es) column for chunk k
                    nc.vector.tensor_scalar(
                        t_t[:, i2 * R : (i2 + 1) * R], p_bc[:],
                        eu2_all[:, ecol : ecol + 1],
                        eu_all[:, ecol : ecol + 1],
                        op0=ALU.mult, op1=ALU.max)
                w_t = pf.tile([P, GRP * R], dt.bfloat16, tag="wt", bufs=2)
                nc.vector.tensor_tensor(
                    w_t[:], t_t[:], ct_g[g][:].rearrange("p i r -> p (i r)"),
                    op=ALU.mult)
                for i in reversed(range(GRP)):
                    k = g * GRP + i
                    off = i * R
                    for n in range(2):
                        nc.tensor.matmul(
                            att_ps[n][:], zfc[:, i, :],
                            w_t[:, off + n * 512 : off + (n + 1) * 512],
                            start=(k == kfirst), stop=(k == klast))
                        nc.tensor.matmul(
                            s_ps[n][:], ident_sb[:],
                            w_t[:, off + n * 512 : off + (n + 1) * 512],
                            start=(k == kfirst), stop=(k == klast))
            for n in range(2):
                nc.scalar.activation(
                    attU_sb[:, n * 512 : (n + 1) * 512], att_ps[n][:], ACT.Relu)
                nc.scalar.activation(
                    S_sb[:, n * 512 : (n + 1) * 512], s_ps[n][:], ACT.Copy)

        # ================= attention finish + gated pooling ==================
        with tc.tile_pool(name="phC", bufs=1) as pg:
            # denominators: column sums of S via colsel matmuls -> [8, 128]
            with tc.tile_pool(name="phC1_ps", bufs=1, space="PSUM") as pg1_ps:
                dT_ps = pg1_ps.tile([RT, P], dt.float32, tag="dT", bufs=1)
                for r in range(RT):
                    nc.tensor.matmul(
                        dT_ps[:], colsel_sb[:, r * RT : (r + 1) * RT],
                        S_sb[:, r * P : (r + 1) * P],
                        start=(r == 0), stop=(r == RT - 1))
                dT_sb = pg.tile([RT, P], dt.bfloat16)
                nc.scalar.activation(dT_sb[:], dT_ps[:], ACT.Copy)
                dcol_ps = pg1_ps.tile([P, RT], dt.bfloat16, tag="dcol", bufs=1)
                nc.tensor.transpose(dcol_ps[:], dT_sb[:], ident_sb[0:RT, 0:RT])
                recd = pg.tile([P, RT], dt.float32)
                nc.vector.reciprocal(recd[:], dcol_ps[:])

            # abU^T = Wab^T @ attU  [2*D_ATT, v]
            abU_sb = pg.tile([P, R], dt.bfloat16)
            with tc.tile_pool(name="phC2_ps", bufs=1, space="PSUM") as pg2_ps:
                for n in range(2):
                    ab_ps = pg2_ps.tile([P, 512], dt.float32, tag="ab", bufs=2)
                    nc.tensor.matmul(
                        ab_ps[:], wab_sb[:],
                        attU_sb[:, n * 512 : (n + 1) * 512],
                        start=True, stop=True)
                    nc.scalar.activation(
                        abU_sb[:, n * 512 : (n + 1) * 512], ab_ps[:], ACT.Copy)

            # transpose per v-block; normalize by 1/denom
            pg_ps = ctx.enter_context(
                tc.tile_pool(name="phC3_ps", bufs=1, space="PSUM"))
            outex = [None] * RT
            abv = pg.tile([P, RT * 2 * D_ATT], dt.float32)
            abv_r = abv.rearrange("p (v t) -> p v t", t=2 * D_ATT)
            for vt in range(RT):
                o_ps = pg_ps.tile([P, P], dt.bfloat16, tag="tr", bufs=6,
                                  name="ops")
                nc.tensor.transpose(
                    o_ps[:], attU_sb[:, vt * P : (vt + 1) * P], ident_sb[:])
                ox = pg.tile([P, O1], dt.bfloat16, tag=f"ox{vt}", bufs=1)
                nc.vector.tensor_scalar(
                    ox[:, 0:OUT], o_ps[:], recd[:, vt : vt + 1], None,
                    op0=ALU.mult)
                nc.vector.memset(ox[:, OUT:O1], 1.0)
                outex[vt] = ox
                a_ps = pg_ps.tile([P, P], dt.bfloat16, tag="tr", bufs=6,
                                  name="aps")
                nc.tensor.transpose(
                    a_ps[:], abU_sb[:, vt * P : (vt + 1) * P], ident_sb[:])
                nc.vector.scalar_tensor_tensor(
                    abv_r[:, vt, :], a_ps[:], recd[:, vt : vt + 1], babbc_sb[:],
                    op0=ALU.mult, op1=ALU.add)

            # gated attention scores: batched tanh / sigmoid / product
            tg = pg.tile([P, RT * 2 * D_ATT], dt.float32)
            tg_r = tg.rearrange("p (v t) -> p v t", t=2 * D_ATT)
            nc.scalar.activation(
                tg_r[:, :, 0:D_ATT], abv_r[:, :, 0:D_ATT], ACT.Tanh)
            nc.scalar.activation(
                tg_r[:, :, D_ATT : 2 * D_ATT],
                abv_r[:, :, D_ATT : 2 * D_ATT], ACT.Sigmoid)
            prods = pg.tile([P, RT * D_ATT], dt.float32)
            prods_r = prods.rearrange("p (v t) -> p v t", t=D_ATT)
            nc.vector.tensor_tensor(
                prods_r[:], tg_r[:, :, 0:D_ATT], tg_r[:, :, D_ATT : 2 * D_ATT],
                op=ALU.mult)
            expa = [None] * RT
            for vt in range(RT):
                junk = pg.tile([P, D_ATT], dt.float32, tag="junk", bufs=2)
                acol = pg.tile([P, 1], dt.float32, tag="acol", bufs=2)
                nc.vector.scalar_tensor_tensor(
                    junk[:], prods_r[:, vt, :], 1.0, wcbc_sb[:],
                    op0=ALU.mult, op1=ALU.mult, accum_out=acol[:])
                ea = pg.tile([P, 1], dt.bfloat16, tag=f"ea{vt}", bufs=1)
                nc.scalar.activation(ea[:], acol[:], ACT.Exp, bias=bcbc_sb[:])
                expa[vt] = ea

            # pooled [1, 129] = sum_v expa_v * [out_v | 1]; AllReduce; divide
            pool_ps = pg_ps.tile([1, O1], dt.float32, tag="pool", bufs=1)
            for vt in range(RT):
                nc.tensor.matmul(
                    pool_ps[:], expa[vt][:], outex[vt][:],
                    start=(vt == 0), stop=(vt == RT - 1))
            pool_sb = pg.tile([1, O1], dt.float32)
            nc.vector.tensor_copy(pool_sb[:], pool_ps[:])
            nc.scalar.dma_start(poolb[:], pool_sb[:])
            nc.gpsimd.collective_compute(
                "AllReduce", ALU.add, replica_groups=groups,
                ins=[poolb[:]], outs=[poolr[:]])
            polr_sb = pg.tile([1, O1], dt.float32)
            nc.scalar.dma_start(polr_sb[:], poolr[:])
            rec2_sb = pg.tile([1, 1], dt.float32)
            nc.vector.reciprocal(rec2_sb[:], polr_sb[:, OUT:O1])
            res_sb = pg.tile([1, OUT], dt.float32)
            nc.vector.tensor_scalar(
                res_sb[:], polr_sb[:, 0:OUT], rec2_sb[:], None, op0=ALU.mult)
            nc.scalar.dma_start(out_ext[:], res_sb[:])

    nc.finalize()
    return nc


_PROGRAM = None


def _get_program():
    global _PROGRAM
    if _PROGRAM is None:
        _PROGRAM = build_program()
    return _PROGRAM


def prep_in_maps(x, G, src, dst, W1, b1, W2, b2, Wg, a_src, a_dst, Wa, ba, Wb, bb,
                 Wc, bc):
    x = np.asarray(x, np.float32)
    G = np.asarray(G, np.float32)
    src = np.asarray(src).astype(np.int64)
    dst = np.asarray(dst).astype(np.int64)

    GT = np.ascontiguousarray(G.T).astype(BF16)
    xT = np.ascontiguousarray(x.T).astype(BF16)
    C = np.zeros((N, N), np.float32)
    np.add.at(C, (src, dst), 1.0)
    C[np.arange(N), np.arange(N)] += 1.0
    Cb = C.astype(BF16)

    va = (np.asarray(Wg, np.float32) @ np.asarray(a_src, np.float32))
    vb = (np.asarray(Wg, np.float32) @ np.asarray(a_dst, np.float32))
    wgv = np.concatenate([np.asarray(Wg, np.float32),
                          np.stack([va, vb], 1)], 1).astype(BF16)
    wab = np.concatenate([np.asarray(Wa, np.float32),
                          np.asarray(Wb, np.float32)], 1).astype(BF16)
    bab = np.concatenate([np.asarray(ba, np.float32),
                          np.asarray(bb, np.float32)], 0)

    colsel = np.zeros((P, RT * RT), np.float32)
    for r in range(RT):
        colsel[:, r * RT + r] = 1.0

    bcast = lambda v: np.broadcast_to(np.asarray(v, np.float32)[None, :],
                                      (P, len(np.asarray(v).reshape(-1)))).copy()
    common = {
        "w1": np.asarray(W1, np.float32).astype(BF16),
        "w2": np.asarray(W2, np.float32).astype(BF16),
        "wgv": wgv,
        "wab": wab,
        "b1bc": bcast(b1),
        "b2bc": bcast(b2),
        "babbc": bcast(bab),
        "wcbc": bcast(np.asarray(Wc, np.float32).reshape(-1)),
        "bcbc": np.full((P, 1), float(np.asarray(bc).reshape(-1)[0]), np.float32),
        "ident": np.eye(P, dtype=np.float32).astype(BF16),
        "colsel": colsel.astype(BF16),
    }
    in_maps = []
    for c in range(M):
        sl = slice(c * R, (c + 1) * R)
        in_maps.append({
            "gt": np.ascontiguousarray(GT[:, sl]),
            "ct": np.ascontiguousarray(Cb[:, sl]),
            "xt": np.ascontiguousarray(xT[:, sl]),
            **common,
        })
    return in_maps


def kernel(**inputs):
    from concourse.bass_utils import run_bass_kernel_spmd

    in_maps = prep_in_maps(**inputs)
    nc = _get_program()
    res = run_bass_kernel_spmd(nc, in_maps, list(range(M)))
    return np.asarray(res.results[0]["out"], np.float32)


# revision 27
# speedup vs baseline: 1.1463x; 1.1463x over previous
"""Trainium2 Bass kernel for nn_H2GT_HGNN: two HGNN convs + single-head GAT +
gated attention pooling, sharded across 8 NeuronCores.

Strategy (dst-node row sharding, core c owns rows R_c = c*1024..(c+1)*1024):
  - Host passes G^T (bf16) so both G @ X convs read contiguous column blocks.
    G^T block is loaded ONCE into SBUF (16 MB resident) and reused by both
    convs; the edge-mask C block streams during the attention phase only.
  - GAT edge softmax is reformulated dense + per-node-rank-1.  Dividing the
    softmax numerator and denominator by ev_v = exp(ed_v) (cancels):
        w[u,v] = C[u,v] * max(eu_u, eu2_u * p_v)
    with eu=exp(es), eu2=exp(0.2 es), p=exp(-0.8 ed).  Per u-chunk this is
    ONE tensor_scalar (4x DVE mode) + ONE tensor_tensor (2x mode), then a
    transposed matmul attT += zchunk^T @ w (stationary z, moving w) plus an
    identity-matmul S += I^T @ w whose column sums give the denominator.
  - Collectives: tiny warmup AllGather first (absorbs comm init), AllGather
    of XW1 / HW2 / [z rows | es/ed rows], final AllReduce of [1,129].
"""

import numpy as np
import ml_dtypes

BF16 = ml_dtypes.bfloat16

# Problem sizes (hardcoded per contract)
N, F_IN, HID, OUT, D_ATT, E = 8192, 512, 256, 128, 64, 262144
M = 8          # cores
P = 128        # partitions
R = N // M     # rows per core (1024)
K = N // P     # contraction chunks (64)
RT = R // P    # row tiles per core (8)
O1 = OUT + 1   # out plus ones column (pooling)
GRP = 4        # chunks per big-matrix DMA group
NG = K // GRP  # groups (16)
ZR = R + 2 * RT  # zeb rows: R z-rows + 16 es/ed rows (1040)


def build_program():
    import concourse.bass as bass
    import concourse.tile as tile
    from concourse import bacc, mybir
    from contextlib import ExitStack

    dt = mybir.dt
    ALU = mybir.AluOpType
    ACT = mybir.ActivationFunctionType

    nc = bacc.Bacc(None, target_bir_lowering=False, debug=False)

    # ---- I/O ----
    gt = nc.dram_tensor("gt", [N, R], dt.bfloat16, kind="ExternalInput")
    ct = nc.dram_tensor("ct", [N, R], dt.bfloat16, kind="ExternalInput")
    xt = nc.dram_tensor("xt", [F_IN, R], dt.bfloat16, kind="ExternalInput")
    w1 = nc.dram_tensor("w1", [F_IN, HID], dt.bfloat16, kind="ExternalInput")
    w2 = nc.dram_tensor("w2", [HID, OUT], dt.bfloat16, kind="ExternalInput")
    wgv = nc.dram_tensor("wgv", [OUT, OUT + 2], dt.bfloat16, kind="ExternalInput")
    wab = nc.dram_tensor("wab", [OUT, 2 * D_ATT], dt.bfloat16, kind="ExternalInput")
    b1bc = nc.dram_tensor("b1bc", [P, HID], dt.float32, kind="ExternalInput")
    b2bc = nc.dram_tensor("b2bc", [P, OUT], dt.float32, kind="ExternalInput")
    babbc = nc.dram_tensor("babbc", [P, 2 * D_ATT], dt.float32, kind="ExternalInput")
    wcbc = nc.dram_tensor("wcbc", [P, D_ATT], dt.float32, kind="ExternalInput")
    bcbc = nc.dram_tensor("bcbc", [P, 1], dt.float32, kind="ExternalInput")
    ident = nc.dram_tensor("ident", [P, P], dt.bfloat16, kind="ExternalInput")
    colsel = nc.dram_tensor("colsel", [P, RT * RT], dt.bfloat16, kind="ExternalInput")
    out_ext = nc.dram_tensor("out", [1, OUT], dt.float32, kind="ExternalOutput")

    groups = [list(range(M))]

    with tile.TileContext(nc) as tc, ExitStack() as ctx:
        dram = ctx.enter_context(tc.tile_pool(name="dram", bufs=1, space="DRAM"))
        xw1b = dram.tile([R, HID], dt.bfloat16)
        xw1f = dram.tile([N, HID], dt.bfloat16, addr_space="Shared")
        hw2b = dram.tile([R, OUT], dt.bfloat16)
        hw2f0 = dram.tile([N // 2, OUT], dt.bfloat16, addr_space="Shared")
        hw2f1 = dram.tile([N // 2, OUT], dt.bfloat16, addr_space="Shared")
        zeb = dram.tile([ZR, OUT], dt.bfloat16)
        zef = dram.tile([M * ZR, OUT], dt.bfloat16, addr_space="Shared")
        poolb = dram.tile([1, O1], dt.float32)
        poolr = dram.tile([1, O1], dt.float32, addr_space="Shared")

        const = ctx.enter_context(tc.tile_pool(name="const", bufs=1))
        big = ctx.enter_context(tc.tile_pool(name="big", bufs=1))

        # ---- resident constants (x^T load is emitted first, inside the
        # gt-pool block below, so it leads the scalar DMA queue) ----
        w1_sb = const.tile([P, (F_IN // P) * HID], dt.bfloat16)
        nc.scalar.dma_start(w1_sb.rearrange("p (k h) -> p k h", h=HID),
                            w1.rearrange("(k p) h -> p k h", p=P))
        w2_sb = const.tile([P, (HID // P) * OUT], dt.bfloat16)
        nc.scalar.dma_start(w2_sb.rearrange("p (k h) -> p k h", h=OUT),
                            w2.rearrange("(k p) h -> p k h", p=P))
        wgv_sb = const.tile([P, OUT + 2], dt.bfloat16)
        nc.scalar.dma_start(wgv_sb[:], wgv[:])
        wab_sb = const.tile([P, 2 * D_ATT], dt.bfloat16)
        nc.scalar.dma_start(wab_sb[:], wab[:])
        b1bc_sb = const.tile([P, HID], dt.float32)
        nc.scalar.dma_start(b1bc_sb[:], b1bc[:])
        b2bc_sb = const.tile([P, OUT], dt.float32)
        nc.scalar.dma_start(b2bc_sb[:], b2bc[:])
        babbc_sb = const.tile([P, 2 * D_ATT], dt.float32)
        nc.scalar.dma_start(babbc_sb[:], babbc[:])
        wcbc_sb = const.tile([P, D_ATT], dt.float32)
        nc.scalar.dma_start(wcbc_sb[:], wcbc[:])
        bcbc_sb = const.tile([P, 1], dt.float32)
        nc.scalar.dma_start(bcbc_sb[:], bcbc[:])
        ident_sb = const.tile([P, P], dt.bfloat16)
        nc.scalar.dma_start(ident_sb[:], ident[:])
        colsel_sb = const.tile([P, RT * RT], dt.bfloat16)
        nc.scalar.dma_start(colsel_sb[:], colsel[:])
        ones_sb = const.tile([1, P], dt.bfloat16)
        nc.vector.memset(ones_sb[:], 1.0)

        # ---- G^T resident load: 16 groups of 4 chunks (sync DMA queue).
        # The pool closes after phase B2 so the C-mask stream can reuse the
        # space during phase D.
        h2T_sb = big.tile([P, R], dt.bfloat16)
        hT_sb = big.tile([P, 2 * R], dt.bfloat16)  # [hid 2x128, r=1024]
        with tc.tile_pool(name="gtp", bufs=1) as gt_pool:
            xt_pool = tc.tile_pool(name="xtp", bufs=1)
            xt_po = xt_pool.__enter__()
            xt_sb = xt_po.tile([P, (F_IN // P) * R], dt.bfloat16)
            nc.scalar.dma_start(xt_sb.rearrange("p (k r) -> p k r", r=R),
                                xt.rearrange("(k p) r -> p k r", p=P))
            # Dummy read of the AG1 output gates the sync DMA queue: the
            # bulk G^T prefetch stays off the HBM bus until the first
            # collective (whose firmware init runs at a fixed early time)
            # completes -- concurrent bulk traffic stretches the init 3x.
            warm_sb = gt_pool.tile([M, 4], dt.bfloat16)
            nc.sync.dma_start(warm_sb[:], xw1f[0:M, 0:4])
            gt_g = []
            for g in range(NG):
                t = gt_pool.tile([P, GRP, R], dt.bfloat16, name=f"gt{g}")
                nc.sync.dma_start(
                    t[:], gt.rearrange("(g i p) r -> g p i r", i=GRP, p=P)[g])
                gt_g.append(t)

            # ============= phase A: XW1 rows R_c, then AllGather =============
            with tc.tile_pool(name="phA", bufs=1) as pa, \
                 tc.tile_pool(name="phA_ps", bufs=1, space="PSUM") as pa_ps:
                xw1_sb = pa.tile([P, RT, HID], dt.bfloat16)
                for rt in range(RT):
                    ps = pa_ps.tile([P, HID], dt.float32, tag="pa", bufs=2)
                    for k in range(F_IN // P):
                        nc.tensor.matmul(
                            ps[:],
                            xt_sb[:, k * R + rt * P : k * R + (rt + 1) * P],
                            w1_sb[:, k * HID : (k + 1) * HID],
                            start=(k == 0), stop=(k == F_IN // P - 1))
                    nc.vector.tensor_tensor(xw1_sb[:, rt, :], ps[:],
                                            b1bc_sb[:], op=ALU.add)
                nc.scalar.dma_start(
                    xw1b.rearrange("(rt p) h -> p rt h", p=P), xw1_sb[:])
            nc.gpsimd.collective_compute(
                "AllGather", ALU.bypass, replica_groups=groups,
                ins=[xw1b[:]], outs=[xw1f[:]])
            xt_pool.__exit__(None, None, None)

            # ============= phase B1: hT = relu(G @ XW1)^T ====================
            # Output-column split: the n=0 half of hT finishes first, so the
            # first half of HW2 (row tiles 0-3) and its AllGather run on the
            # CC queue while the PE computes the n=1 half.
            with tc.tile_pool(name="phB1", bufs=1) as pb, \
                 tc.tile_pool(name="phB1_ps", bufs=1, space="PSUM") as pb_ps, \
                 tc.tile_pool(name="phB1b_ps", bufs=1, space="PSUM") as pbb_ps:
                hps = [pb_ps.tile([P, 512], dt.float32, tag=f"h{j}{n}", bufs=1,
                                  name=f"hps{j}{n}")
                       for j in range(2) for n in range(2)]
                xw1cs = []
                for g in range(NG):
                    xw1c = pb.tile([P, GRP, HID], dt.bfloat16, name=f"xw1c{g}")
                    nc.scalar.dma_start(
                        xw1c[:],
                        xw1f.rearrange("(g i p) h -> g p i h", i=GRP, p=P)[g])
                    xw1cs.append(xw1c)
                hw2_sb = pb.tile([P, RT, OUT], dt.bfloat16)

                def b1_half(n):
                    for g in range(NG):
                        for i in range(GRP):
                            k = g * GRP + i
                            gsl = gt_g[g][:, i, :]
                            for j in range(2):
                                nc.tensor.matmul(
                                    hps[2 * j + n][:],
                                    xw1cs[g][:, i, j * P : (j + 1) * P],
                                    gsl[:, n * 512 : (n + 1) * 512],
                                    start=(k == 0), stop=(k == K - 1))
                    for j in range(2):
                        nc.scalar.activation(
                            hT_sb[:, j * R + n * 512 : j * R + (n + 1) * 512],
                            hps[2 * j + n][:], ACT.Relu)

                def hw2_part(half):
                    for rt in range(4 * half, 4 * half + 4):
                        ps2 = pbb_ps.tile([P, OUT], dt.float32, tag="hw2",
                                          bufs=2)
                        for j in range(2):
                            nc.tensor.matmul(
                                ps2[:],
                                hT_sb[:, j * R + rt * P : j * R + (rt + 1) * P],
                                w2_sb[:, j * OUT : (j + 1) * OUT],
                                start=(j == 0), stop=(j == 1))
                        nc.vector.tensor_tensor(hw2_sb[:, rt, :], ps2[:],
                                                b2bc_sb[:], op=ALU.add)
                    nc.scalar.dma_start(
                        hw2b.rearrange("(hf rt p) h -> hf p rt h",
                                       hf=2, p=P)[half],
                        hw2_sb[:, 4 * half : 4 * half + 4, :])

                hw2cs = {}

                def b2_dmas(half, hw2fh, pool):
                    for c in range(M):
                        hw2c = pool.tile([P, GRP, OUT], dt.bfloat16,
                                         name=f"hw2c{2 * c + half}")
                        nc.scalar.dma_start(
                            hw2c[:],
                            hw2fh.rearrange("(c i p) h -> c p i h",
                                            i=GRP, p=P)[c])
                        hw2cs[2 * c + half] = hw2c

                b1_half(0)
                hw2_part(0)
                nc.gpsimd.collective_compute(
                    "AllGather", ALU.bypass, replica_groups=groups,
                    ins=[hw2b[0 : R // 2, :]], outs=[hw2f0[:]])
                b2_dmas(0, hw2f0, pb)
                b1_half(1)
                hw2_part(1)
                nc.gpsimd.collective_compute(
                    "AllGather", ALU.bypass, replica_groups=groups,
                    ins=[hw2b[R // 2 : R, :]], outs=[hw2f1[:]])
                b2_dmas(1, hw2f1, pb)

                # ========= phase B2: h2T = (G @ HW2)^T =======================
                # Contraction split: u-chunks covered by hw2f0 (m=0..3 of
                # every core) are accumulated first; the second half-AllGather
                # hides under them.
                with tc.tile_pool(name="phB2_ps", bufs=1,
                                  space="PSUM") as pc_ps:
                    h2ps = [pc_ps.tile([P, 512], dt.float32, tag=f"h2{n}",
                                       bufs=1, name=f"h2ps{n}")
                            for n in range(2)]
                    for half in range(2):
                        for c in range(M):
                            g = 2 * c + half
                            for i in range(GRP):
                                gsl = gt_g[g][:, i, :]
                                for n in range(2):
                                    nc.tensor.matmul(
                                        h2ps[n][:], hw2cs[g][:, i, :],
                                        gsl[:, n * 512 : (n + 1) * 512],
                                        start=(half == 0 and c == 0
                                               and i == 0),
                                        stop=(half == 1 and c == M - 1
                                              and i == GRP - 1))
                    for n in range(2):
                        nc.scalar.activation(
                            h2T_sb[:, n * 512 : (n + 1) * 512], h2ps[n][:],
                            ACT.Copy)

        # ---- z rows + es/ed, packed into one AllGather buffer ----
        esed_col = big.tile([P, 2 * RT], dt.bfloat16)  # [v-part, (rt: es, ed)]
        with tc.tile_pool(name="phB2b", bufs=1) as pd, \
             tc.tile_pool(name="phB2b_ps", bufs=1, space="PSUM") as pd_ps:
            zrow = pd.tile([P, RT, OUT], dt.bfloat16)
            for rt in range(RT):
                zps = pd_ps.tile([P, OUT + 2], dt.float32, tag="z", bufs=2)
                nc.tensor.matmul(
                    zps[:], h2T_sb[:, rt * P : (rt + 1) * P], wgv_sb[:],
                    start=True, stop=True)
                nc.scalar.activation(zrow[:, rt, :], zps[:, 0:OUT], ACT.Copy)
                nc.vector.tensor_copy(esed_col[:, 2 * rt : 2 * rt + 2],
                                      zps[:, OUT : OUT + 2])
            nc.scalar.dma_start(
                zeb[0:R, :].rearrange("(rt p) h -> p rt h", p=P), zrow[:])
            # es/ed rows (local, transposed) appended below the z rows
            eT_ps = pd_ps.tile([2 * RT, P], dt.bfloat16, tag="eT", bufs=1)
            nc.tensor.transpose(eT_ps[:], esed_col[:], ident_sb[:])
            eT_sb = pd.tile([2 * RT, P], dt.bfloat16)
            nc.scalar.activation(eT_sb[:], eT_ps[:], ACT.Copy)
            nc.scalar.dma_start(zeb[R : ZR, :], eT_sb[:])

        nc.gpsimd.collective_compute(
            "AllGather", ALU.bypass, replica_groups=groups,
            ins=[zeb[:]], outs=[zef[:]])

        zef_r = zef.rearrange("(c q) f -> c q f", q=ZR)

        # ================= phase D prep ======================================
        # eu_all/eu2_all hold exp(es), exp(0.2 es) for ALL nodes in columns
        # m*16 + 2c (odd columns hold exp of ed -- unused junk).
        eu_all = big.tile([P, 2 * K], dt.float32)
        eu2_all = big.tile([P, 2 * K], dt.float32)
        with tc.tile_pool(name="phDp", bufs=1) as pe, \
             tc.tile_pool(name="phDp_ps", bufs=1, space="PSUM") as pe_ps:
            # eu/eu2 for all nodes: one gather of every es/ed row, 8 packed
            # transposes into one PSUM tile, then two full-width exps.
            est_all = pe.tile([2 * M, RT * P], dt.bfloat16)
            for m in range(RT):
                nc.scalar.dma_start(
                    est_all[:, m * P : (m + 1) * P]
                    .rearrange("(c j) f -> c j f", j=2),
                    zef_r[:, R + 2 * m : R + 2 * m + 2, :])
            estT_ps = pe_ps.tile([P, 2 * K], dt.bfloat16, tag="estT", bufs=1)
            for m in range(RT):
                nc.tensor.transpose(
                    estT_ps[:, 2 * M * m : 2 * M * (m + 1)],
                    est_all[:, m * P : (m + 1) * P],
                    ident_sb[0:2 * M, 0:2 * M])
            nc.scalar.activation(eu_all[:], estT_ps[:], ACT.Exp)
            nc.scalar.activation(eu2_all[:], estT_ps[:], ACT.Exp, scale=0.2)

        # z chunks for the attention stationaries
        zfc_pool = ctx.enter_context(tc.tile_pool(name="zfcp", bufs=1))
        zfcs = {}
        for g in [2 * c + half for half in range(2) for c in range(M)]:
            half, c = g % 2, g // 2
            zfc = zfc_pool.tile([P, GRP, OUT], dt.bfloat16, name=f"zfc{g}")
            nc.scalar.dma_start(
                zfc[:],
                zef_r[c, half * GRP * P : (half * GRP + GRP) * P, :]
                .rearrange("(i p) f -> p i f", p=P))
            zfcs[g] = zfc

        # p_bc[u, v] = exp(-0.8 * ed_v): built from LOCAL ed only, emitted
        # before the AllGathers so it overlaps them.
        p_bc = big.tile([P, R], dt.bfloat16)
        with tc.tile_pool(name="phPb", bufs=1) as pe0, \
             tc.tile_pool(name="phPb_ps", bufs=1, space="PSUM") as pe0_ps:
            for m in range(RT):
                edr_ps = pe0_ps.tile([1, P], dt.bfloat16, tag="edr", bufs=2)
                nc.tensor.transpose(
                    edr_ps[:], esed_col[:, 2 * m + 1 : 2 * m + 2], ident_sb[:])
                edr_sb = pe0.tile([1, P], dt.bfloat16, tag="edrs", bufs=2)
                nc.vector.tensor_copy(edr_sb[:], edr_ps[:])
                bc_ps = pe0_ps.tile([P, P], dt.float32, tag="bc", bufs=2)
                nc.tensor.matmul(bc_ps[:], ones_sb[:], edr_sb[:],
                                 start=True, stop=True)
                nc.scalar.activation(
                    p_bc[:, m * P : (m + 1) * P], bc_ps[:], ACT.Exp, scale=-0.8)

        # C mask groups stream into the space freed by gt (sync queue);
        # gated behind the last AllGather (dummy read of its output) so their
        # HBM traffic cannot slow the collectives.
        ct_pool = ctx.enter_context(tc.tile_pool(name="ctp", bufs=1))
        ct_gate = ct_pool.tile([M, 4], dt.bfloat16)
        nc.sync.dma_start(ct_gate[:], zef[0:M, 0:4])
        ct_g = [None] * NG
        for g in [2 * c + half for half in range(2) for c in range(M)]:
            t = ct_pool.tile([P, GRP, R], dt.bfloat16, name=f"ct{g}")
            nc.sync.dma_start(
                t[:], ct.rearrange("(g i p) r -> g p i r", i=GRP, p=P)[g])
            ct_g[g] = t

        # ================= phase D loop ======================================
        # DVE computes t (tensor_scalar, 2x mode) then the mask product
        # (tensor_tensor, 2x mode); GPSIMD is deliberately NOT used -- a
        # concurrent GpSimd op halves DVE throughput (SBUF contention).
        # Matmuls are issued in reversed order within groups of GRP chunks
        # so the PE fires 16 back-to-back matmuls per group (a >=3.4us
        # burst holds the HAM clock warm).
        attU_sb = big.tile([P, R], dt.bfloat16)   # relu(att)^T  [f, v]
        S_sb = big.tile([P, R], dt.bfloat16)      # sum_k w_k    [p, v]
        with tc.tile_pool(name="phD", bufs=1) as pf, \
             tc.tile_pool(name="phD_ps", bufs=1, space="PSUM") as pf_ps:
            att_ps = [pf_ps.tile([P, 512], dt.float32, tag=f"att{n}", bufs=1,
                                 name=f"attps{n}") for n in range(2)]
            s_ps = [pf_ps.tile([P, 512], dt.float32, tag=f"s{n}", bufs=1,
                               name=f"sps{n}") for n in range(2)]
            gorder = [2 * c + half for half in range(2) for c in range(M)]
            korder = []
            for g in gorder:
                korder.extend(reversed(range(g * GRP, (g + 1) * GRP)))
            kfirst, klast = korder[0], korder[-1]
            for g in gorder:
                zfc = zfcs[g]
                t_t = pf.tile([P, GRP * R], dt.bfloat16, tag="tt", bufs=2)
                for i2 in range(GRP):
                    k = g * GRP + i2
                    c, m = k // RT, k % RT
                    ecol = 2 * M * m + 2 * c  # exp(es) column for chunk k
                    nc.vector.tensor_scalar(
                        t_t[:, i2 * R : (i2 + 1) * R], p_bc[:],
                        eu2_all[:, ecol : ecol + 1],
                        eu_all[:, ecol : ecol + 1],
                        op0=ALU.mult, op1=ALU.max)
                w_t = pf.tile([P, GRP * R], dt.bfloat16, tag="wt", bufs=2)
                nc.vector.tensor_tensor(
                    w_t[:], t_t[:], ct_g[g][:].rearrange("p i r -> p (i r)"),
                    op=ALU.mult)
                for i in reversed(range(GRP)):
                    k = g * GRP + i
                    off = i * R
                    for n in range(2):
                        nc.tensor.matmul(
                            att_ps[n][:], zfc[:, i, :],
                            w_t[:, off + n * 512 : off + (n + 1) * 512],
                            start=(k == kfirst), stop=(k == klast))
                        nc.tensor.matmul(
                            s_ps[n][:], ident_sb[:],
                            w_t[:, off + n * 512 : off + (n + 1) * 512],
                            start=(k == kfirst), stop=(k == klast))
            for n in range(2):
                nc.scalar.activation(
                    attU_sb[:, n * 512 : (n + 1) * 512], att_ps[n][:], ACT.Relu)
                nc.scalar.activation(
                    S_sb[:, n * 512 : (n + 1) * 512], s_ps[n][:], ACT.Copy)

        # ================= attention finish + gated pooling ==================
        with tc.tile_pool(name="phC", bufs=1) as pg:
            # denominators: column sums of S via colsel matmuls -> [8, 128]
            with tc.tile_pool(name="phC1_ps", bufs=1, space="PSUM") as pg1_ps:
                dT_ps = pg1_ps.tile([RT, P], dt.float32, tag="dT", bufs=1)
                for r in range(RT):
                    nc.tensor.matmul(
                        dT_ps[:], colsel_sb[:, r * RT : (r + 1) * RT],
                        S_sb[:, r * P : (r + 1) * P],
                        start=(r == 0), stop=(r == RT - 1))
                dT_sb = pg.tile([RT, P], dt.bfloat16)
                nc.scalar.activation(dT_sb[:], dT_ps[:], ACT.Copy)
                dcol_ps = pg1_ps.tile([P, RT], dt.bfloat16, tag="dcol", bufs=1)
                nc.tensor.transpose(dcol_ps[:], dT_sb[:], ident_sb[0:RT, 0:RT])
                recd = pg.tile([P, RT], dt.float32)
                nc.vector.reciprocal(recd[:], dcol_ps[:])

            # abU^T = Wab^T @ attU  [2*D_ATT, v]
            abU_sb = pg.tile([P, R], dt.bfloat16)
            with tc.tile_pool(name="phC2_ps", bufs=1, space="PSUM") as pg2_ps:
                for n in range(2):
                    ab_ps = pg2_ps.tile([P, 512], dt.float32, tag="ab", bufs=2)
                    nc.tensor.matmul(
                        ab_ps[:], wab_sb[:],
                        attU_sb[:, n * 512 : (n + 1) * 512],
                        start=True, stop=True)
                    nc.scalar.activation(
                        abU_sb[:, n * 512 : (n + 1) * 512], ab_ps[:], ACT.Copy)

            # transpose per v-block; normalize by 1/denom
            pg_ps = ctx.enter_context(
                tc.tile_pool(name="phC3_ps", bufs=1, space="PSUM"))
            outex = [None] * RT
            abv = pg.tile([P, RT * 2 * D_ATT], dt.float32)
            abv_r = abv.rearrange("p (v t) -> p v t", t=2 * D_ATT)
            for vt in range(RT):
                o_ps = pg_ps.tile([P, P], dt.bfloat16, tag="tr", bufs=6,
                                  name="ops")
                nc.tensor.transpose(
                    o_ps[:], attU_sb[:, vt * P : (vt + 1) * P], ident_sb[:])
                ox = pg.tile([P, O1], dt.bfloat16, tag=f"ox{vt}", bufs=1)
                nc.vector.tensor_scalar(
                    ox[:, 0:OUT], o_ps[:], recd[:, vt : vt + 1], None,
                    op0=ALU.mult)
                nc.vector.memset(ox[:, OUT:O1], 1.0)
                outex[vt] = ox
                a_ps = pg_ps.tile([P, P], dt.bfloat16, tag="tr", bufs=6,
                                  name="aps")
                nc.tensor.transpose(
                    a_ps[:], abU_sb[:, vt * P : (vt + 1) * P], ident_sb[:])
                nc.vector.scalar_tensor_tensor(
                    abv_r[:, vt, :], a_ps[:], recd[:, vt : vt + 1], babbc_sb[:],
                    op0=ALU.mult, op1=ALU.add)

            # gated attention scores: batched tanh / sigmoid / product
            tg = pg.tile([P, RT * 2 * D_ATT], dt.float32)
            tg_r = tg.rearrange("p (v t) -> p v t", t=2 * D_ATT)
            nc.scalar.activation(
                tg_r[:, :, 0:D_ATT], abv_r[:, :, 0:D_ATT], ACT.Tanh)
            nc.scalar.activation(
                tg_r[:, :, D_ATT : 2 * D_ATT],
                abv_r[:, :, D_ATT : 2 * D_ATT], ACT.Sigmoid)
            prods = pg.tile([P, RT * D_ATT], dt.float32)
            prods_r = prods.rearrange("p (v t) -> p v t", t=D_ATT)
            nc.vector.tensor_tensor(
                prods_r[:], tg_r[:, :, 0:D_ATT], tg_r[:, :, D_ATT : 2 * D_ATT],
                op=ALU.mult)
            expa = [None] * RT
            for vt in range(RT):
                junk = pg.tile([P, D_ATT], dt.float32, tag="junk", bufs=2)
                acol = pg.tile([P, 1], dt.float32, tag="acol", bufs=2)
                nc.vector.scalar_tensor_tensor(
                    junk[:], prods_r[:, vt, :], 1.0, wcbc_sb[:],
                    op0=ALU.mult, op1=ALU.mult, accum_out=acol[:])
                ea = pg.tile([P, 1], dt.bfloat16, tag=f"ea{vt}", bufs=1)
                nc.scalar.activation(ea[:], acol[:], ACT.Exp, bias=bcbc_sb[:])
                expa[vt] = ea

            # pooled [1, 129] = sum_v expa_v * [out_v | 1]; AllReduce; divide
            pool_ps = pg_ps.tile([1, O1], dt.float32, tag="pool", bufs=1)
            for vt in range(RT):
                nc.tensor.matmul(
                    pool_ps[:], expa[vt][:], outex[vt][:],
                    start=(vt == 0), stop=(vt == RT - 1))
            pool_sb = pg.tile([1, O1], dt.float32)
            nc.vector.tensor_copy(pool_sb[:], pool_ps[:])
            nc.scalar.dma_start(poolb[:], pool_sb[:])
            nc.gpsimd.collective_compute(
                "AllReduce", ALU.add, replica_groups=groups,
                ins=[poolb[:]], outs=[poolr[:]])
            polr_sb = pg.tile([1, O1], dt.float32)
            nc.scalar.dma_start(polr_sb[:], poolr[:])
            rec2_sb = pg.tile([1, 1], dt.float32)
            nc.vector.reciprocal(rec2_sb[:], polr_sb[:, OUT:O1])
            res_sb = pg.tile([1, OUT], dt.float32)
            nc.vector.tensor_scalar(
                res_sb[:], polr_sb[:, 0:OUT], rec2_sb[:], None, op0=ALU.mult)
            nc.scalar.dma_start(out_ext[:], res_sb[:])

    nc.finalize()
    return nc


_PROGRAM = None


def _get_program():
    global _PROGRAM
    if _PROGRAM is None:
        _PROGRAM = build_program()
    return _PROGRAM


def prep_in_maps(x, G, src, dst, W1, b1, W2, b2, Wg, a_src, a_dst, Wa, ba, Wb, bb,
                 Wc, bc):
    x = np.asarray(x, np.float32)
    G = np.asarray(G, np.float32)
    src = np.asarray(src).astype(np.int64)
    dst = np.asarray(dst).astype(np.int64)

    GT = np.ascontiguousarray(G.T).astype(BF16)
    xT = np.ascontiguousarray(x.T).astype(BF16)
    C = np.zeros((N, N), np.float32)
    np.add.at(C, (src, dst), 1.0)
    C[np.arange(N), np.arange(N)] += 1.0
    Cb = C.astype(BF16)

    va = (np.asarray(Wg, np.float32) @ np.asarray(a_src, np.float32))
    vb = (np.asarray(Wg, np.float32) @ np.asarray(a_dst, np.float32))
    wgv = np.concatenate([np.asarray(Wg, np.float32),
                          np.stack([va, vb], 1)], 1).astype(BF16)
    wab = np.concatenate([np.asarray(Wa, np.float32),
                          np.asarray(Wb, np.float32)], 1).astype(BF16)
    bab = np.concatenate([np.asarray(ba, np.float32),
                          np.asarray(bb, np.float32)], 0)

    colsel = np.zeros((P, RT * RT), np.float32)
    for r in range(RT):
        colsel[:, r * RT + r] = 1.0

    bcast = lambda v: np.broadcast_to(np.asarray(v, np.float32)[None, :],
                                      (P, len(np.asarray(v).reshape(-1)))).copy()
    common = {
        "w1": np.asarray(W1, np.float32).astype(BF16),
        "w2": np.asarray(W2, np.float32).astype(BF16),
        "wgv": wgv,
        "wab": wab,
        "b1bc": bcast(b1),
        "b2bc": bcast(b2),
        "babbc": bcast(bab),
        "wcbc": bcast(np.asarray(Wc, np.float32).reshape(-1)),
        "bcbc": np.full((P, 1), float(np.asarray(bc).reshape(-1)[0]), np.float32),
        "ident": np.eye(P, dtype=np.float32).astype(BF16),
        "colsel": colsel.astype(BF16),
    }
    in_maps = []
    for c in range(M):
        sl = slice(c * R, (c + 1) * R)
        in_maps.append({
            "gt": np.ascontiguousarray(GT[:, sl]),
            "ct": np.ascontiguousarray(Cb[:, sl]),
            "xt": np.ascontiguousarray(xT[:, sl]),
            **common,
        })
    return in_maps


def kernel(**inputs):
    from concourse.bass_utils import run_bass_kernel_spmd

    in_maps = prep_in_maps(**inputs)
    nc = _get_program()
    res = run_bass_kernel_spmd(nc, in_maps, list(range(M)))
    return np.asarray(res.results[0]["out"], np.float32)
